# revision 1
# baseline (speedup 1.0000x reference)
"""Trainium2 Bass kernel for nn_HausdorffDistance_28406913696124.

Math (reference):
    px = (prob_map[0].ravel() >= 0.5)                 # [N], N = 100*100
    py = (gt_map.ravel()   >= 0.5)                    # [N]
    D[i,j] = euclid dist between grid points i, j     # [N, N] constant!
    loss   = mean_i | px_i * mean_j D[i,j] - (D @ py)_i / N |

Key structure: D depends only on (|r_i-r_j|, |c_i-c_j|) with r=i//100,
c=i%100.  So:
  * rowmean_i = mean_j D[i,j] is a pure constant -> precomputed on host.
  * (D @ py) is a 2D correlation of the 100x100 binary mask PY with the
    199x199 kernel sqrt(dr^2+dc^2).  Factor it through the distance table
    Q[u,v] = sqrt(u^2+v^2), u,v in [0,100):

        term2sum[r,c] = sum_d ( H_d[r-d, c] + H_d[r+d, c] )   (d=0 once)
        H_d  = PY @ T_d          T_d[b,c] = Q[d, |b-c|]   (Toeplitz)

    On the PE the +-d row shifts are folded into the stationary operand:
    for the 13 d's owned by a core (d = 13k + j, j = 0..12) the stationary
    C_j[b, r] = PYT_plus[b, r-j] + PYT_minus[b, r+j], where PYT_plus/minus
    are the transposed binary mask pre-shifted by +-13k (shift baked into
    the per-core input data, so the SPMD program only uses j = 0..12 as
    compile-time AP offsets).  One strided DVE add builds all 13 C_j from
    zero-padded tiles; 13 accumulating matmuls then produce this core's
    partial term2sum [100,100] directly in PSUM.

Sharding: 100 d-values split across 8 cores (13/core, zero padded), an
on-device AllReduce sums the partial term2sum maps, then every core
computes the identical final scalar.
"""

import sys

import numpy as np

sys.path.insert(0, "/opt/trn_rl_repo")

H = 100
N = H * H
NCORES = 8
DSH = 13   # d-values per core (8*13 = 104 >= 100, rest zero-padded)
PADW = 13  # zero pad on each side of the transposed-mask tiles
CHUNK = 500  # free-dim chunk (matmul <= 512 fp32 PSUM bank)


def _host_constants():
    """Geometry-only constant tables (input independent)."""
    idx = np.arange(H)
    absdiff = np.abs(idx[:, None] - idx[None, :])  # [100,100] |b-c|
    # fp32-exact integer squares -> correctly rounded fp32 sqrt: matches the
    # reference's gram-matrix + sqrt exactly.
    q32 = np.sqrt((idx[:, None] ** 2 + idx[None, :] ** 2).astype(np.float32))

    # rowsum[r,c] = sum_j D[i,j] (i = r*100+c), accumulated in float64.
    # (N * rowmean -- the 1/N^2 is folded into the final scalar scale.)
    cnt = np.zeros((H, H))  # cnt[r,u] = #{a : |r-a| = u}
    np.add.at(cnt, (idx[:, None], absdiff), 1.0)
    # negated so the device can fold "- px*rowsumN" into the 9-way
    # gather-sum (see _build_module).
    rowsumN = (-(cnt @ q32.astype(np.float64) @ cnt.T)).astype(np.float32)

    q16 = q32.astype(np.float16)
    t_slices = []
    for k in range(NCORES):
        t_k = np.zeros((H, DSH * H), dtype=np.float16)
        for j in range(DSH):
            d = k * DSH + j
            if d >= H:
                continue
            blk = q16[d, absdiff]
            if d == 0:
                # d=0 appears in both the +j and -j branch of the combined
                # stationary; halve once so it is counted once.
                blk = (blk.astype(np.float32) * 0.5).astype(np.float16)
            t_k[:, j * H:(j + 1) * H] = blk
        t_slices.append(t_k)
    return rowsumN, t_slices


def _build_module(with_collective=True):
    import concourse.bacc as bacc
    import concourse.mybir as mybir
    import concourse.tile as tile

    f32 = mybir.dt.float32
    f16 = mybir.dt.float16

    nc = bacc.Bacc(
        "TRN2",
        target_bir_lowering=False,
        debug=False,
        enable_asserts=False,
        num_devices=NCORES,
    )

    # gtpack = gtT_plus | gtT_minus ; rmprob = rowsumN | prob  ([100,100] f32)
    gtpack_d = nc.dram_tensor("gtpack", [H, 2 * H], f32, kind="ExternalInput")
    rmprob_d = nc.dram_tensor("rmprob", [H, 2 * H], f32, kind="ExternalInput")
    tsl_d = nc.dram_tensor("t_slice", [H, DSH * H], f16, kind="ExternalInput")
    out_d = nc.dram_tensor("out", [1, 1], f32, kind="ExternalOutput")

    PW = H + 2 * PADW  # padded width of the transposed-mask tiles

    with tile.TileContext(nc) as tc:
        with (
            tc.tile_pool(name="sb", bufs=1) as sb,
            tc.tile_pool(name="ps_acc", bufs=1, space="PSUM") as ps_acc,
            tc.tile_pool(name="ps_fin", bufs=1, space="PSUM") as ps_fin,
            tc.tile_pool(name="dram", bufs=1, space="DRAM") as dram,
        ):
            # ---- loads (gt/rm on ACT ring, T on SP ring; a single
            # InstDMACopy is split across all 16 SDMA engines on HW) ------
            gtpack_sb = sb.tile([H, 2 * H], f32)
            nc.scalar.dma_start(gtpack_sb[:], gtpack_d[:])
            gtp_sb = gtpack_sb[:, 0:H]
            gtm_sb = gtpack_sb[:, H:2 * H]
            rmprob_sb = sb.tile([H, 2 * H], f32)
            nc.scalar.dma_start(rmprob_sb[:], rmprob_d[:])
            rm_sb = rmprob_sb[:, 0:H]
            prob_sb = rmprob_sb[:, H:2 * H]

            tsl_sb = sb.tile([H, DSH * H], f16)
            nc.sync.dma_start(tsl_sb[:], tsl_d[:])

            # ---- binarize the pre-shifted transposed masks -------------
            pytp = sb.tile([H, PW], f16)  # PYT_plus, zero padded
            pytm = sb.tile([H, PW], f16)  # PYT_minus, zero padded
            nc.vector.memset(pytp[:], 0.0)
            nc.vector.memset(pytm[:], 0.0)
            nc.vector.tensor_scalar(
                pytp[:, PADW:PADW + H], gtp_sb, 0.5, None, mybir.AluOpType.is_ge
            )
            nc.vector.tensor_scalar(
                pytm[:, PADW:PADW + H], gtm_sb, 0.5, None, mybir.AluOpType.is_ge
            )

            # ---- combined stationary: C_j[b, m] = pytp[b, PADW-j+m]
            #                                     + pytm[b, PADW+j+m] ----
            comb = sb.tile([H, DSH * H], f16)
            for j in range(DSH):
                nc.vector.tensor_add(
                    comb[:, j * H:(j + 1) * H],
                    pytp[:, PADW - j:PADW - j + H],
                    pytm[:, PADW + j:PADW + j + H],
                )

            # ---- 13 accumulating matmuls -> partial term2sum in PSUM ---
            acc_ps = ps_acc.tile([H, H], f32)
            for j in range(DSH):
                nc.tensor.matmul(
                    acc_ps[:],
                    comb[:, j * H:(j + 1) * H],
                    tsl_sb[:, j * H:(j + 1) * H],
                    start=(j == 0),
                    stop=(j == DSH - 1),
                )
            # ---- AllGather the 8 partial maps, sum them on-device ------
            # (AG floor ~5us vs AR ~10us on 8 cores; the 8-way sum is one
            # strided DVE reduce over a [100, 100, 8] view.)
            part2 = sb.tile([H, H], f32)
            nc.vector.tensor_copy(part2[:], acc_ps[:])
            cc_in = dram.tile([H, H], f32)
            cc_out = dram.tile([NCORES * H, H], f32, addr_space="Shared")
            nc.sync.dma_start(cc_in[:], part2[:])
            if with_collective:
                nc.gpsimd.collective_compute(
                    "AllGather",
                    mybir.AluOpType.bypass,
                    replica_groups=[list(range(NCORES))],
                    ins=[cc_in[:].opt()],
                    outs=[cc_out[:].opt()],
                )
                gath_src = cc_out[:]
            else:
                # timing-model variant (no collectives in sim): fake the
                # gather with a single same-sized DRAM read.
                gath_src = cc_out[:]
            # gath slices g=0..7: the gathered partial maps; slice 8:
            # t1n = px * (-rowsumN).  One strided 9-way reduce then gives
            # diff = term2sum - px*rowsumN directly.
            gath = sb.tile([H, (NCORES + 1) * H], f32)
            nc.vector.scalar_tensor_tensor(
                gath[:, NCORES * H:(NCORES + 1) * H],
                prob_sb,
                0.5,
                rm_sb,
                op0=mybir.AluOpType.is_ge,
                op1=mybir.AluOpType.mult,
            )
            # DRAM [g*H + p, c] -> SBUF [p, g*H + c]
            nc.scalar.dma_start(
                gath[:, 0:NCORES * H].rearrange("p (g c) -> p g c", g=NCORES),
                gath_src.rearrange("(g p) c -> p g c", g=NCORES),
            )
            diff = sb.tile([H, H], f32)
            nc.vector.tensor_reduce(
                diff[:],
                gath[:].rearrange("p (g c) -> p c g", g=NCORES + 1),
                axis=mybir.AxisListType.X,
                op=mybir.AluOpType.add,
            )
            rowsums = sb.tile([H, 1], f32)
            nc.vector.tensor_reduce(
                rowsums[:],
                diff[:],
                axis=mybir.AxisListType.X,
                op=mybir.AluOpType.add,
                apply_absolute_value=True,
            )
            ones_sb = sb.tile([H, 1], f32)
            nc.vector.memset(ones_sb[:], 1.0)
            fin_ps = ps_fin.tile([1, 1], f32)
            nc.tensor.matmul(fin_ps[:], rowsums[:], ones_sb[:])
            out_sb = sb.tile([1, 1], f32)
            nc.vector.tensor_scalar_mul(out_sb[:], fin_ps[:], 1.0 / (N * N))
            nc.sync.dma_start(out_d[:], out_sb[:])

    nc.compile()
    return nc


_STATE = {}


def _get_state():
    if not _STATE:
        rowsumN, t_slices = _host_constants()
        _STATE["consts"] = (rowsumN, t_slices)
        _STATE["nc"] = _build_module()
    return _STATE


def _in_maps(prob_map, gt_map):
    st = _get_state()
    rowsumN, t_slices = st["consts"]
    prob = np.asarray(prob_map, dtype=np.float32).reshape(H, H)
    gt = np.asarray(gt_map, dtype=np.float32).reshape(H, H)
    gtT = np.ascontiguousarray(gt.T)

    rmprob = np.ascontiguousarray(np.concatenate([rowsumN, prob], axis=1))
    in_maps = []
    for k in range(NCORES):
        dk = k * DSH
        gtp = np.zeros((H, H), dtype=np.float32)
        gtm = np.zeros((H, H), dtype=np.float32)
        gtp[:, dk:] = gtT[:, :H - dk]
        gtm[:, :H - dk] = gtT[:, dk:]
        gtpack = np.ascontiguousarray(np.concatenate([gtp, gtm], axis=1))
        in_maps.append(
            {"gtpack": gtpack, "rmprob": rmprob, "t_slice": t_slices[k]}
        )
    return in_maps


def _run(prob_map, gt_map, trace=False, **spmd_kwargs):
    from concourse import bass_utils

    st = _get_state()
    in_maps = _in_maps(prob_map, gt_map)
    res = bass_utils.run_bass_kernel_spmd(
        st["nc"], in_maps, core_ids=list(range(NCORES)), trace=trace,
        **spmd_kwargs,
    )
    value = np.float32(res.results[0]["out"][0, 0])
    return value, res


def kernel(prob_map, gt_map):
    value, _ = _run(prob_map, gt_map, trace=False)
    return np.asarray(value, dtype=np.float32)



# revision 12
# speedup vs baseline: 2.0569x; 2.0569x over previous
"""Trainium2 Bass kernel for nn_HausdorffDistance_28406913696124.

Math (reference):
    px = (prob_map[0].ravel() >= 0.5)                 # [N], N = 100*100
    py = (gt_map.ravel()   >= 0.5)                    # [N]
    D[i,j] = euclid dist between grid points i, j     # [N, N] constant!
    loss   = mean_i | px_i * mean_j D[i,j] - (D @ py)_i / N |

Key structure: D depends only on (|r_i-r_j|, |c_i-c_j|), so

  * rowmean_i = mean_j D[i,j] is a pure constant -> precomputed on host.
  * (D @ py) is the 2D correlation of the binary mask PY with the radial
    kernel Q[u,v] = sqrt(u^2+v^2), u,v in [0,100).  A rank-R truncated
    (multiplicity-weighted) SVD  Q ~= sum_k a_k b_k^T  makes the
    correlation separable:

        term2sum = sum_k  A_k @ PY @ B_k,
        A_k[r,r'] = a_k[|r-r'|],  B_k[c,c'] = b_k[|c-c'|]   (sym Toeplitz)

    R = 4 already gives ~1e-6 relative error on the final scalar
    (tolerance 2e-2): the |.| + mean over 10^4 pixels buries both the
    truncation and the fp16 rounding noise.

On device this is two matmul stages on a SINGLE core (no collective):
    stage 1:  U[r', k*100+c] = sum_c' PYT[c',r'] * Bcat[c',k*100+c]
              (one matmul, stationary = binarized transposed mask)
    stage 2:  R accumulating matmuls  term2 += A_k^T @ U_k, seeded by
              writing t1n = px * (-rowsumN) into the PSUM bank first so
              the accumulation directly yields  diff = term2sum - px*rowsumN.
Tail: abs-row-reduce, 1/N^2-scaled ones matmul -> scalar, DMA out.
"""

import sys

import numpy as np

sys.path.insert(0, "/opt/trn_rl_repo")

H = 100
N = H * H
R = 2          # SVD rank of the distance kernel
# Scale folded into Acat + rowsumN so no on-device final scaling is
# needed; 2^-13 is mantissa-exact in f16/f32.  The host multiplies the
# returned raw |diff| total by 1 / (N^2 * SCALE) when unpacking.
SCALE = 2.0 ** -13
SEED_PSUM = True  # write t1n into the stage-2 PSUM bank, matmuls accumulate


def _host_constants():
    """Geometry-only constant tables (input independent)."""
    idx = np.arange(H)
    absdiff = np.abs(idx[:, None] - idx[None, :])  # [100,100] |b-c|
    q = np.sqrt((idx[:, None] ** 2 + idx[None, :] ** 2).astype(np.float64))

    # rowsum[r,c] = sum_j D[i,j] (i = r*100+c) in float64, negated so the
    # device can seed the PSUM accumulation with px * (-rowsumN), and
    # pre-scaled by SCALE (matching Acat) so no device-side scaling of
    # the final reduction is needed.
    cnt = np.zeros((H, H))  # cnt[r,u] = #{a : |r-a| = u}
    np.add.at(cnt, (idx[:, None], absdiff), 1.0)
    rowsumn = (-SCALE * (cnt @ q @ cnt.T)).astype(np.float32)

    # multiplicity-weighted rank-R SVD of Q (weights = how often each
    # (u,v) displacement occurs in the 100x100 grid)
    m = np.where(idx == 0, 100.0, 2.0 * (100 - idx))
    sw = np.sqrt(m)
    uu, ss, vt = np.linalg.svd(sw[:, None] * q * sw[None, :])
    a = (uu[:, :R] * np.sqrt(ss[:R])) / sw[:, None]
    b = (vt[:R, :].T * np.sqrt(ss[:R])) / sw[:, None]

    # Bcat[c', k*100+c] = b_k[|c'-c|];  Acat[r', k*100+r] = a_k[|r'-r|]
    # (Acat carries the SCALE factor; 2^-13 keeps f16 mantissas exact.)
    bcat = np.concatenate(
        [b[absdiff, k] for k in range(R)], axis=1).astype(np.float16)
    acat = np.concatenate(
        [SCALE * a[absdiff, k] for k in range(R)], axis=1).astype(np.float16)
    return rowsumn, bcat, acat


def _build_module():
    import concourse.bacc as bacc
    import concourse.mybir as mybir
    import concourse.tile as tile

    f32 = mybir.dt.float32
    f16 = mybir.dt.float16

    nc = bacc.Bacc(
        "TRN2",
        target_bir_lowering=False,
        debug=False,
        enable_asserts=False,
        num_devices=1,
    )

    # One packed input: Bcat | Acat | rowsumN_neg(f32 as 2xf16) | gtT-.5
    # | prob-.5   ([100, 2R*100 + 400] f16).  A single DMA instruction
    # pays the fixed HWDGE(625) + dge-delay(650) + sem-prop(900) once.
    PK = 2 * R * H + 4 * H
    pack_d = nc.dram_tensor("pack", [H, PK], f16, kind="ExternalInput")
    out_d = nc.dram_tensor("out", [1, 1], f32, kind="ExternalOutput")

    with tile.TileContext(nc) as tc:
        with (
            tc.tile_pool(name="sb", bufs=1) as sb,
            tc.tile_pool(name="ps_u", bufs=1, space="PSUM") as ps_u,
            tc.tile_pool(name="ps_t2", bufs=1, space="PSUM") as ps_t2,
        ):
            pack_sb = sb.tile([H, PK], f16)
            nc.sync.dma_start(pack_sb[:], pack_d[:])
            bcat_sb = pack_sb[:, 0:R * H]
            acat_sb = pack_sb[:, R * H:2 * R * H]
            rsn_sb = pack_sb[:, 2 * R * H:2 * R * H + 2 * H].bitcast(f32)
            gtt_sb = pack_sb[:, 2 * R * H + 2 * H:2 * R * H + 3 * H]
            prob_sb = pack_sb[:, 2 * R * H + 3 * H:PK]

            # ---- binarize the transposed mask (f16 0/1) -----------------
            pyt = sb.tile([H, H], f16)
            nc.vector.tensor_scalar(
                pyt[:], gtt_sb, 0.0, None, mybir.AluOpType.is_ge
            )

            # ---- t1n = (prob >= .5) * (-rowsumN), written to the stage-2
            #      PSUM bank so the matmuls accumulate on top of it -------
            t2_ps = ps_t2.tile([H, H], f32)
            t1n_dst = t2_ps[:] if SEED_PSUM else None
            if not SEED_PSUM:
                t1n_sb = sb.tile([H, H], f32)
                t1n_dst = t1n_sb[:]
            nc.vector.scalar_tensor_tensor(
                t1n_dst,
                prob_sb,
                0.0,
                rsn_sb,
                op0=mybir.AluOpType.is_ge,
                op1=mybir.AluOpType.mult,
            )

            # ---- stage 1: U = PYT^T @ Bcat  (one matmul, N = R*100) -----
            u_ps = ps_u.tile([H, R * H], f32)
            nc.tensor.matmul(u_ps[:], pyt[:], bcat_sb, start=True, stop=True)

            # PSUM -> SBUF (downcast f16) on the ACT engine
            u_sb = sb.tile([H, R * H], f16)
            nc.scalar.activation(
                u_sb[:], u_ps[:], mybir.ActivationFunctionType.Copy
            )

            # ---- stage 2: diff = t1n + sum_k A_k^T @ U_k ----------------
            for k in range(R):
                nc.tensor.matmul(
                    t2_ps[:],
                    acat_sb[:, k * H:(k + 1) * H],
                    u_sb[:, k * H:(k + 1) * H],
                    start=(k == 0 and not SEED_PSUM),
                    stop=(k == R - 1),
                    skip_group_check=SEED_PSUM,
                )
            if not SEED_PSUM:
                diff_sb = sb.tile([H, H], f32)
                nc.vector.tensor_add(diff_sb[:], t2_ps[:], t1n_sb[:])
                red_src = diff_sb[:]
            else:
                red_src = t2_ps[:]

            # ---- tail: sum_i |diff_i| (the 1/(N^2*SCALE) is host-side) --
            import concourse.bass_isa as bass_isa
            rowabs = sb.tile([H, 1], f32)
            nc.vector.tensor_reduce(
                rowabs[:],
                red_src,
                axis=mybir.AxisListType.X,
                op=mybir.AluOpType.add,
                apply_absolute_value=True,
            )
            tot = sb.tile([H, 1], f32)
            nc.gpsimd.partition_all_reduce(
                tot[:], rowabs[:], channels=H, reduce_op=bass_isa.ReduceOp.add
            )
            nc.sync.dma_start(out_d[:], tot[0:1, 0:1])

    nc.compile()
    return nc


_STATE = {}


def _get_state():
    if not _STATE:
        rowsumn, bcat, acat = _host_constants()
        pack = np.empty((H, 2 * R * H + 4 * H), dtype=np.float16)
        pack[:, 0:R * H] = bcat
        pack[:, R * H:2 * R * H] = acat
        pack[:, 2 * R * H:2 * R * H + 2 * H] = rowsumn.view(np.float16)
        _STATE["pack"] = pack
        _STATE["nc"] = _build_module()
    return _STATE


def _in_maps(prob_map, gt_map):
    st = _get_state()
    pack = st["pack"]
    prob = np.asarray(prob_map, dtype=np.float32).reshape(H, H)
    gt = np.asarray(gt_map, dtype=np.float32).reshape(H, H)
    # x - 0.5 is sign-exact in f32; the f16 cast can only flip the
    # comparison for |x-0.5| < 2^-25 (measure ~3e-8 per element).
    pack[:, 2 * R * H + 2 * H:2 * R * H + 3 * H] = (
        gt.T - np.float32(0.5)).astype(np.float16)
    pack[:, 2 * R * H + 3 * H:] = (prob - np.float32(0.5)).astype(np.float16)
    return [{"pack": np.ascontiguousarray(pack)}]


def _run(prob_map, gt_map, trace=False, **spmd_kwargs):
    from concourse import bass_utils

    st = _get_state()
    in_maps = _in_maps(prob_map, gt_map)
    res = bass_utils.run_bass_kernel_spmd(
        st["nc"], in_maps, core_ids=[0], trace=trace, **spmd_kwargs,
    )
    raw = np.float64(res.results[0]["out"][0, 0])
    value = np.float32(raw / (SCALE * N * N))
    return value, res


def kernel(prob_map, gt_map):
    value, _ = _run(prob_map, gt_map, trace=False)
    return np.asarray(value, dtype=np.float32)


# revision 26
# speedup vs baseline: 2.3543x; 1.1446x over previous
"""Trainium2 Bass kernel for nn_HausdorffDistance_28406913696124.

Math (reference):
    px = (prob_map[0].ravel() >= 0.5)                 # [N], N = 100*100
    py = (gt_map.ravel()   >= 0.5)                    # [N]
    D[i,j] = euclid dist between grid points i, j     # [N, N] constant!
    loss   = mean_i | px_i * mean_j D[i,j] - (D @ py)_i / N |

Key structure: D depends only on (|r_i-r_j|, |c_i-c_j|), so

  * rowmean_i = mean_j D[i,j] is a pure constant -> precomputed on host.
  * (D @ py) is the 2D correlation of the binary mask PY with the radial
    kernel Q[u,v] = sqrt(u^2+v^2), u,v in [0,100).  A rank-R truncated
    (multiplicity-weighted) SVD  Q ~= sum_k a_k b_k^T  makes the
    correlation separable:

        term2sum = sum_k  A_k @ PY @ B_k,
        A_k[r,r'] = a_k[|r-r'|],  B_k[c,c'] = b_k[|c-c'|]   (sym Toeplitz)

    R = 4 already gives ~1e-6 relative error on the final scalar
    (tolerance 2e-2): the |.| + mean over 10^4 pixels buries both the
    truncation and the fp16 rounding noise.

On device this is two matmul stages on a SINGLE core (no collective):
    stage 1:  U[r', k*100+c] = sum_c' PYT[c',r'] * Bcat[c',k*100+c]
              (one matmul, stationary = binarized transposed mask)
    stage 2:  R accumulating matmuls  term2 += A_k^T @ U_k, seeded by
              writing t1n = px * (-rowsumN) into the PSUM bank first so
              the accumulation directly yields  diff = term2sum - px*rowsumN.
Tail: abs-row-reduce, 1/N^2-scaled ones matmul -> scalar, DMA out.
"""

import sys

import numpy as np

sys.path.insert(0, "/opt/trn_rl_repo")

H = 100
N = H * H
R = 2          # SVD rank of the distance kernel
# Scale folded into Acat + rowsumN so no on-device final scaling is
# needed; 2^-13 is mantissa-exact in f16/f32.  The host multiplies the
# returned raw |diff| total by 1 / (N^2 * SCALE) when unpacking.
SCALE = 2.0 ** -13
SEED_PSUM = True  # write t1n into the stage-2 PSUM bank, matmuls accumulate


def _host_constants():
    """Geometry-only constant tables (input independent)."""
    idx = np.arange(H)
    absdiff = np.abs(idx[:, None] - idx[None, :])  # [100,100] |b-c|
    q = np.sqrt((idx[:, None] ** 2 + idx[None, :] ** 2).astype(np.float64))

    # rowsum[r,c] = sum_j D[i,j] (i = r*100+c) in float64, negated so the
    # device can seed the PSUM accumulation with px * (-rowsumN), and
    # pre-scaled by SCALE (matching Acat) so no device-side scaling of
    # the final reduction is needed.
    cnt = np.zeros((H, H))  # cnt[r,u] = #{a : |r-a| = u}
    np.add.at(cnt, (idx[:, None], absdiff), 1.0)
    rowsumn = (-SCALE * (cnt @ q @ cnt.T)).astype(np.float32)

    # multiplicity-weighted rank-R SVD of Q (weights = how often each
    # (u,v) displacement occurs in the 100x100 grid)
    m = np.where(idx == 0, 100.0, 2.0 * (100 - idx))
    sw = np.sqrt(m)
    uu, ss, vt = np.linalg.svd(sw[:, None] * q * sw[None, :])
    a = (uu[:, :R] * np.sqrt(ss[:R])) / sw[:, None]
    b = (vt[:R, :].T * np.sqrt(ss[:R])) / sw[:, None]

    # Bcat[c', k*100+c] = b_k[|c'-c|];  Acat[r', k*100+r] = a_k[|r'-r|]
    # (Acat carries the SCALE factor; 2^-13 keeps f16 mantissas exact.)
    bcat = np.concatenate(
        [b[absdiff, k] for k in range(R)], axis=1).astype(np.float16)
    acat = np.concatenate(
        [SCALE * a[absdiff, k] for k in range(R)], axis=1).astype(np.float16)
    return rowsumn, bcat, acat


def _build_module():
    import concourse.bacc as bacc
    import concourse.mybir as mybir
    import concourse.tile as tile

    f32 = mybir.dt.float32
    f16 = mybir.dt.float16

    nc = bacc.Bacc(
        "TRN2",
        target_bir_lowering=False,
        debug=False,
        enable_asserts=False,
        num_devices=1,
    )

    # One packed input: Bcat | Acat | rowsumN_neg(f32 as 2xf16) | gtT-.5
    # | prob-.5   ([100, 2R*100 + 400] f16).  A single DMA instruction
    # pays the fixed HWDGE(625) + dge-delay(650) + sem-prop(900) once.
    PK = 2 * R * H + 4 * H
    pack_d = nc.dram_tensor("pack", [H, PK], f16, kind="ExternalInput")
    # dma_scatter_add payload granularity is 256B = 64 f32; the scalar
    # result lands in out[0, 0], the rest is junk the host ignores.
    out_d = nc.dram_tensor("out", [1, 64], f32, kind="ExternalOutput")

    with tile.TileContext(nc) as tc:
        with (
            tc.tile_pool(name="sb", bufs=1) as sb,
            tc.tile_pool(name="ps_u", bufs=1, space="PSUM") as ps_u,
            tc.tile_pool(name="ps_t2", bufs=1, space="PSUM") as ps_t2,
            tc.tile_pool(name="ps_fin", bufs=1, space="PSUM") as ps_fin,
        ):
            pack_sb = sb.tile([H, PK], f16)
            nc.sync.dma_start(pack_sb[:], pack_d[:])

            # ---- output path setup, all off the critical path -----------
            # A single-descriptor dma_scatter_add (index 0) ships
            # pay[0, 0:64] to the output.  Descriptors are PREPARED early
            # on the idle Pool engine so firing them later only costs a
            # trigger + the completion-sem latency (saves the 625ns HWDGE
            # + 650ns DGE delay of a plain store).  The runner pre-zeros
            # ExternalOutput buffers, so the += lands on zeros.
            from concourse import library_config
            nc.gpsimd.load_library(library_config.mlp)
            idx_sb = sb.tile([128, 1], mybir.dt.int16)
            nc.vector.memset(idx_sb[:], 0)
            pay = sb.tile([128, 64], f32)
            nc.vector.memset(pay[:], 0.0)
            out_sem = nc.alloc_semaphore("out_dma")
            nc.gpsimd.dma_scatter_add(
                out_d[:],
                pay[:].rearrange("p (a b) -> p a b", a=1),
                idx_sb[:],
                1,
                1,
                64,
                prepare_only=True,
                sem=out_sem,
            )
            bcat_sb = pack_sb[:, 0:R * H]
            acat_sb = pack_sb[:, R * H:2 * R * H]
            rsn_sb = pack_sb[:, 2 * R * H:2 * R * H + 2 * H].bitcast(f32)
            gtt_sb = pack_sb[:, 2 * R * H + 2 * H:2 * R * H + 3 * H]
            prob_sb = pack_sb[:, 2 * R * H + 3 * H:PK]

            # ---- binarize the transposed mask (f16 0/1) -----------------
            pyt = sb.tile([H, H], f16)
            nc.vector.tensor_scalar(
                pyt[:], gtt_sb, 0.0, None, mybir.AluOpType.is_ge
            )

            # ---- t1n = (prob >= .5) * (-rowsumN), written to the stage-2
            #      PSUM bank so the matmuls accumulate on top of it -------
            t2_ps = ps_t2.tile([H, H], f32)
            t1n_dst = t2_ps[:] if SEED_PSUM else None
            if not SEED_PSUM:
                t1n_sb = sb.tile([H, H], f32)
                t1n_dst = t1n_sb[:]
            nc.vector.scalar_tensor_tensor(
                t1n_dst,
                prob_sb,
                0.0,
                rsn_sb,
                op0=mybir.AluOpType.is_ge,
                op1=mybir.AluOpType.mult,
            )

            # ---- stage 1: U = PYT^T @ Bcat  (one matmul, N = R*100) -----
            u_ps = ps_u.tile([H, R * H], f32)
            nc.tensor.matmul(u_ps[:], pyt[:], bcat_sb, start=True, stop=True)

            # PSUM -> SBUF (downcast f16) on the ACT engine
            u_sb = sb.tile([H, R * H], f16)
            nc.scalar.activation(
                u_sb[:], u_ps[:], mybir.ActivationFunctionType.Copy
            )

            # ---- stage 2: diff = t1n + sum_k A_k^T @ U_k ----------------
            for k in range(R):
                nc.tensor.matmul(
                    t2_ps[:],
                    acat_sb[:, k * H:(k + 1) * H],
                    u_sb[:, k * H:(k + 1) * H],
                    start=(k == 0 and not SEED_PSUM),
                    stop=(k == R - 1),
                    skip_group_check=SEED_PSUM,
                )
            if not SEED_PSUM:
                diff_sb = sb.tile([H, H], f32)
                nc.vector.tensor_add(diff_sb[:], t2_ps[:], t1n_sb[:])
                red_src = diff_sb[:]
            else:
                red_src = t2_ps[:]

            # ---- tail: abs-row-reduce (DVE), cross-partition sum via a
            #      ones-matmul (PE), copy the scalar into the scatter
            #      payload's partition 0, fire the prepared DMA.  (Pool
            #      only ever runs [lib-load, prep, trigger], so the
            #      early prep is never stuck behind late Pool compute.) --
            rowabs = sb.tile([H, 1], f32)
            nc.vector.tensor_reduce(
                rowabs[:],
                red_src,
                axis=mybir.AxisListType.X,
                op=mybir.AluOpType.add,
                apply_absolute_value=True,
            )
            ones_sb = sb.tile([H, 1], f32)
            nc.vector.memset(ones_sb[:], 1.0)
            fin_ps = ps_fin.tile([1, 1], f32)
            nc.tensor.matmul(fin_ps[:], rowabs[:], ones_sb[:])
            nc.vector.tensor_copy(pay[0:1, 0:1], fin_ps[:])
            nc.gpsimd.trigger_dma(count=None)

    # Tile's gen_mode==1 sem plumbing gap: the epilogue gate waits on the
    # DMASW lane semaphore, but a PREPARED descriptor bumps the caller's
    # sem= semaphore instead (routed to on_update[0] at prep time).
    # Retarget the wait at the semaphore the descriptor actually updates.
    dma_upd = None
    for blk in nc.m.functions[0].blocks:
        for inst in blk.instructions:
            if type(inst).__name__ == "InstDMAScatterAddAnt":
                dma_upd = inst.sync_info.on_update[0]
    assert dma_upd is not None and dma_upd.ant_name == "out_dma"
    for blk in nc.m.functions[0].blocks:
        for inst in blk.instructions:
            si = getattr(inst, "sync_info", None)
            if si is None:
                continue
            for w in si.on_wait:
                if str(getattr(w, "ant_name", "")).startswith("DMASW"):
                    w.id = dma_upd.id
                    w.ant_name = dma_upd.ant_name

    nc.compile()
    return nc


_STATE = {}


def _get_state():
    if not _STATE:
        rowsumn, bcat, acat = _host_constants()
        pack = np.empty((H, 2 * R * H + 4 * H), dtype=np.float16)
        pack[:, 0:R * H] = bcat
        pack[:, R * H:2 * R * H] = acat
        pack[:, 2 * R * H:2 * R * H + 2 * H] = rowsumn.view(np.float16)
        _STATE["pack"] = pack
        _STATE["nc"] = _build_module()
    return _STATE


def _in_maps(prob_map, gt_map):
    st = _get_state()
    pack = st["pack"]
    prob = np.asarray(prob_map, dtype=np.float32).reshape(H, H)
    gt = np.asarray(gt_map, dtype=np.float32).reshape(H, H)
    # x - 0.5 is sign-exact in f32; the f16 cast can only flip the
    # comparison for |x-0.5| < 2^-25 (measure ~3e-8 per element).
    pack[:, 2 * R * H + 2 * H:2 * R * H + 3 * H] = (
        gt.T - np.float32(0.5)).astype(np.float16)
    pack[:, 2 * R * H + 3 * H:] = (prob - np.float32(0.5)).astype(np.float16)
    return [{"pack": np.ascontiguousarray(pack)}]


def _run(prob_map, gt_map, trace=False, **spmd_kwargs):
    from concourse import bass_utils

    st = _get_state()
    in_maps = _in_maps(prob_map, gt_map)
    res = bass_utils.run_bass_kernel_spmd(
        st["nc"], in_maps, core_ids=[0], trace=trace, **spmd_kwargs,
    )
    raw = np.float64(res.results[0]["out"][0, 0])
    value = np.float32(raw / (SCALE * N * N))
    return value, res


def kernel(prob_map, gt_map):
    value, _ = _run(prob_map, gt_map, trace=False)
    return np.asarray(value, dtype=np.float32)


# revision 34
# speedup vs baseline: 2.3719x; 1.0075x over previous
"""Trainium2 Bass kernel for nn_HausdorffDistance_28406913696124.

Math (reference):
    px = (prob_map[0].ravel() >= 0.5)                 # [N], N = 100*100
    py = (gt_map.ravel()   >= 0.5)                    # [N]
    D[i,j] = euclid dist between grid points i, j     # [N, N] constant!
    loss   = mean_i | px_i * mean_j D[i,j] - (D @ py)_i / N |

Key structure: D depends only on (|r_i-r_j|, |c_i-c_j|), so

  * rowmean_i = mean_j D[i,j] is a pure constant -> precomputed on host.
  * (D @ py) is the 2D correlation of the binary mask PY with the radial
    kernel Q[u,v] = sqrt(u^2+v^2), u,v in [0,100).  A rank-R truncated
    (multiplicity-weighted) SVD  Q ~= sum_k a_k b_k^T  makes the
    correlation separable:

        term2sum = sum_k  A_k @ PY @ B_k,
        A_k[r,r'] = a_k[|r-r'|],  B_k[c,c'] = b_k[|c-c'|]   (sym Toeplitz)

    R = 4 already gives ~1e-6 relative error on the final scalar
    (tolerance 2e-2): the |.| + mean over 10^4 pixels buries both the
    truncation and the fp16 rounding noise.

On device this is two matmul stages on a SINGLE core (no collective):
    stage 1:  U[r', k*100+c] = sum_c' PYT[c',r'] * Bcat[c',k*100+c]
              (one matmul, stationary = binarized transposed mask)
    stage 2:  R accumulating matmuls  term2 += A_k^T @ U_k, seeded by
              writing t1n = px * (-rowsumN) into the PSUM bank first so
              the accumulation directly yields  diff = term2sum - px*rowsumN.
Tail: abs-row-reduce, 1/N^2-scaled ones matmul -> scalar, DMA out.
"""

import sys

import numpy as np

sys.path.insert(0, "/opt/trn_rl_repo")

H = 100
N = H * H
R = 2          # SVD rank of the distance kernel
# Scale folded into Acat + rowsumN so no on-device final scaling is
# needed; 2^-13 is mantissa-exact in f16/f32.  The host multiplies the
# returned raw |diff| total by 1 / (N^2 * SCALE) when unpacking.
SCALE = 2.0 ** -13
SEED_PSUM = True  # write t1n into the stage-2 PSUM bank, matmuls accumulate


def _host_constants():
    """Geometry-only constant tables (input independent)."""
    idx = np.arange(H)
    absdiff = np.abs(idx[:, None] - idx[None, :])  # [100,100] |b-c|
    q = np.sqrt((idx[:, None] ** 2 + idx[None, :] ** 2).astype(np.float64))

    # rowsum[r,c] = sum_j D[i,j] (i = r*100+c) in float64, negated so the
    # device can seed the PSUM accumulation with px * (-rowsumN), and
    # pre-scaled by SCALE (matching Acat) so no device-side scaling of
    # the final reduction is needed.
    cnt = np.zeros((H, H))  # cnt[r,u] = #{a : |r-a| = u}
    np.add.at(cnt, (idx[:, None], absdiff), 1.0)
    rowsumn = (-SCALE * (cnt @ q @ cnt.T)).astype(np.float32)

    # multiplicity-weighted rank-R SVD of Q (weights = how often each
    # (u,v) displacement occurs in the 100x100 grid)
    m = np.where(idx == 0, 100.0, 2.0 * (100 - idx))
    sw = np.sqrt(m)
    uu, ss, vt = np.linalg.svd(sw[:, None] * q * sw[None, :])
    a = (uu[:, :R] * np.sqrt(ss[:R])) / sw[:, None]
    b = (vt[:R, :].T * np.sqrt(ss[:R])) / sw[:, None]

    # Bcat[c', k*100+c] = b_k[|c'-c|];  Acat[r', k*100+r] = a_k[|r'-r|]
    # (Acat carries the SCALE factor; 2^-13 keeps f16 mantissas exact.)
    bcat = np.concatenate(
        [b[absdiff, k] for k in range(R)], axis=1).astype(np.float16)
    acat = np.concatenate(
        [SCALE * a[absdiff, k] for k in range(R)], axis=1).astype(np.float16)
    return rowsumn, bcat, acat


def _build_module():
    import concourse.bacc as bacc
    import concourse.mybir as mybir
    import concourse.tile as tile

    f32 = mybir.dt.float32
    f16 = mybir.dt.float16

    nc = bacc.Bacc(
        "TRN2",
        target_bir_lowering=False,
        debug=False,
        enable_asserts=False,
        num_devices=1,
    )

    # One packed input: Bcat | Acat | rowsumN_neg(f32 as 2xf16) | gtT-.5
    # | prob-.5   ([100, 2R*100 + 400] f16).  A single DMA instruction
    # pays the fixed HWDGE(625) + dge-delay(650) + sem-prop(900) once.
    PK = 2 * R * H + 4 * H
    pack_d = nc.dram_tensor("pack", [H, PK], f16, kind="ExternalInput")
    # dma_scatter_add payload granularity is 256B = 64 f32; the scalar
    # result lands in out[0, 0], the rest is junk the host ignores.
    out_d = nc.dram_tensor("out", [1, 64], f32, kind="ExternalOutput")

    with tile.TileContext(nc) as tc:
        with (
            tc.tile_pool(name="sb", bufs=1) as sb,
            tc.tile_pool(name="ps_u", bufs=1, space="PSUM") as ps_u,
            tc.tile_pool(name="ps_t2", bufs=1, space="PSUM") as ps_t2,
            tc.tile_pool(name="ps_fin", bufs=1, space="PSUM") as ps_fin,
        ):
            pack_sb = sb.tile([H, PK], f16)
            nc.sync.dma_start(pack_sb[:], pack_d[:])

            # ---- output path setup, all off the critical path -----------
            # A single-descriptor dma_scatter_add (index 0) ships
            # pay[0, 0:64] to the output.  Descriptors are PREPARED early
            # on the idle Pool engine so firing them later only costs a
            # trigger + the completion-sem latency (saves the 625ns HWDGE
            # + 650ns DGE delay of a plain store).  The runner pre-zeros
            # ExternalOutput buffers, so the += lands on zeros.
            from concourse import library_config
            nc.gpsimd.load_library(library_config.mlp)
            idx_sb = sb.tile([128, 1], mybir.dt.int16)
            nc.vector.memset(idx_sb[:], 0)
            pay = sb.tile([128, 64], f32)
            nc.vector.memset(pay[:], 0.0)
            out_sem = nc.alloc_semaphore("out_dma")
            nc.gpsimd.dma_scatter_add(
                out_d[:],
                pay[:].rearrange("p (a b) -> p a b", a=1),
                idx_sb[:],
                1,
                1,
                64,
                prepare_only=True,
                sem=out_sem,
            )
            bcat_sb = pack_sb[:, 0:R * H]
            acat_sb = pack_sb[:, R * H:2 * R * H]
            rsn_sb = pack_sb[:, 2 * R * H:2 * R * H + 2 * H].bitcast(f32)
            gtt_sb = pack_sb[:, 2 * R * H + 2 * H:2 * R * H + 3 * H]
            prob_sb = pack_sb[:, 2 * R * H + 3 * H:PK]

            # ---- binarize the transposed mask (f16 0/1) -----------------
            pyt = sb.tile([H, H], f16)
            nc.vector.tensor_scalar(
                pyt[:], gtt_sb, 0.0, None, mybir.AluOpType.is_ge
            )

            # ---- t1n = (prob >= .5) * (-rowsumN), written to the stage-2
            #      PSUM bank so the matmuls accumulate on top of it -------
            t2_ps = ps_t2.tile([H, H], f32)
            t1n_dst = t2_ps[:] if SEED_PSUM else None
            if not SEED_PSUM:
                t1n_sb = sb.tile([H, H], f32)
                t1n_dst = t1n_sb[:]
            nc.vector.scalar_tensor_tensor(
                t1n_dst,
                prob_sb,
                0.0,
                rsn_sb,
                op0=mybir.AluOpType.is_ge,
                op1=mybir.AluOpType.mult,
            )

            # ---- stage 1: U = PYT^T @ Bcat  (one matmul, N = R*100) -----
            u_ps = ps_u.tile([H, R * H], f32)
            nc.tensor.matmul(u_ps[:], pyt[:], bcat_sb, start=True, stop=True)

            # PSUM -> SBUF (downcast f16) on the ACT engine
            u_sb = sb.tile([H, R * H], f16)
            nc.scalar.activation(
                u_sb[:], u_ps[:], mybir.ActivationFunctionType.Copy
            )

            # ---- stage 2: diff = t1n + sum_k A_k^T @ U_k ----------------
            for k in range(R):
                nc.tensor.matmul(
                    t2_ps[:],
                    acat_sb[:, k * H:(k + 1) * H],
                    u_sb[:, k * H:(k + 1) * H],
                    start=(k == 0 and not SEED_PSUM),
                    stop=(k == R - 1),
                    skip_group_check=SEED_PSUM,
                )
            if not SEED_PSUM:
                diff_sb = sb.tile([H, H], f32)
                nc.vector.tensor_add(diff_sb[:], t2_ps[:], t1n_sb[:])
                red_src = diff_sb[:]
            else:
                red_src = t2_ps[:]

            # ---- tail: abs-row-reduce (DVE), cross-partition sum via a
            #      ones-matmul (PE), copy the scalar into the scatter
            #      payload's partition 0, fire the prepared DMA.  (Pool
            #      only ever runs [lib-load, prep, trigger], so the
            #      early prep is never stuck behind late Pool compute.) --
            rowabs = sb.tile([H, 1], f32)
            nc.vector.tensor_reduce(
                rowabs[:],
                red_src,
                axis=mybir.AxisListType.X,
                op=mybir.AluOpType.add,
                apply_absolute_value=True,
            )
            ones_sb = sb.tile([H, 1], f32)
            nc.vector.memset(ones_sb[:], 1.0)
            fin_ps = ps_fin.tile([1, 1], f32)
            nc.tensor.matmul(fin_ps[:], rowabs[:], ones_sb[:])
            nc.scalar.activation(
                pay[0:1, 0:1], fin_ps[:], mybir.ActivationFunctionType.Copy
            )
            nc.gpsimd.trigger_dma(count=None)

    # Tile's gen_mode==1 sem plumbing gap: the epilogue gate waits on the
    # DMASW lane semaphore, but a PREPARED descriptor bumps the caller's
    # sem= semaphore instead (routed to on_update[0] at prep time).
    # Retarget the wait at the semaphore the descriptor actually updates.
    dma_upd = None
    for blk in nc.m.functions[0].blocks:
        for inst in blk.instructions:
            if type(inst).__name__ == "InstDMAScatterAddAnt":
                dma_upd = inst.sync_info.on_update[0]
    assert dma_upd is not None and dma_upd.ant_name == "out_dma"
    for blk in nc.m.functions[0].blocks:
        for inst in blk.instructions:
            si = getattr(inst, "sync_info", None)
            if si is None:
                continue
            for w in si.on_wait:
                if str(getattr(w, "ant_name", "")).startswith("DMASW"):
                    w.id = dma_upd.id
                    w.ant_name = dma_upd.ant_name

    # The epilogue's two all-engine barrier rounds (~600ns of drains) are
    # independent of the output-DMA completion, but the auto-inserted gate
    # serializes them behind it.  Move the out_dma wait from that gate to
    # the FINAL release broadcaster so the drain ladder overlaps the
    # DMA-completion semaphore latency; every engine's stream still ends
    # strictly after the output write has landed.
    insts = [i for blk in nc.m.functions[0].blocks for i in blk.instructions]
    gate_wait = None
    for inst in insts:
        si = getattr(inst, "sync_info", None)
        if si is None:
            continue
        for w in si.on_wait:
            if w.ant_name == "out_dma":
                gate_wait = w
                si.on_wait = [x for x in si.on_wait if x.ant_name != "out_dma"]
                break
    release = None
    for inst in insts:
        if inst.name.startswith("barrier_Pool"):
            si = inst.sync_info
            if (si and not si.on_wait and si.on_update
                    and "release" in str(si.on_update[0].ant_name)):
                release = inst
    assert gate_wait is not None and release is not None
    release.sync_info.on_wait = list(release.sync_info.on_wait) + [gate_wait]

    nc.compile()
    return nc


_STATE = {}


def _get_state():
    if not _STATE:
        rowsumn, bcat, acat = _host_constants()
        pack = np.empty((H, 2 * R * H + 4 * H), dtype=np.float16)
        pack[:, 0:R * H] = bcat
        pack[:, R * H:2 * R * H] = acat
        pack[:, 2 * R * H:2 * R * H + 2 * H] = rowsumn.view(np.float16)
        _STATE["pack"] = pack
        _STATE["nc"] = _build_module()
    return _STATE


def _in_maps(prob_map, gt_map):
    st = _get_state()
    pack = st["pack"]
    prob = np.asarray(prob_map, dtype=np.float32).reshape(H, H)
    gt = np.asarray(gt_map, dtype=np.float32).reshape(H, H)
    # x - 0.5 is sign-exact in f32; the f16 cast can only flip the
    # comparison for |x-0.5| < 2^-25 (measure ~3e-8 per element).
    pack[:, 2 * R * H + 2 * H:2 * R * H + 3 * H] = (
        gt.T - np.float32(0.5)).astype(np.float16)
    pack[:, 2 * R * H + 3 * H:] = (prob - np.float32(0.5)).astype(np.float16)
    return [{"pack": np.ascontiguousarray(pack)}]


def _run(prob_map, gt_map, trace=False, **spmd_kwargs):
    from concourse import bass_utils

    st = _get_state()
    in_maps = _in_maps(prob_map, gt_map)
    res = bass_utils.run_bass_kernel_spmd(
        st["nc"], in_maps, core_ids=[0], trace=trace, **spmd_kwargs,
    )
    raw = np.float64(res.results[0]["out"][0, 0])
    value = np.float32(raw / (SCALE * N * N))
    return value, res


def kernel(prob_map, gt_map):
    value, _ = _run(prob_map, gt_map, trace=False)
    return np.asarray(value, dtype=np.float32)
